# revision 16
# baseline (speedup 1.0000x reference)
"""Conditional BatchNorm1d (training-mode, per-class stats) on 8 Trainium2
NeuronCores.

Problem: x [512, 128, 1024] f32, labels [512] i32 in [0,8), weight/bias
[8, 128] f32.  Per-class biased mean/var over the class's (batch, length)
elements per feature, then per-class affine:
    y = x * (rsqrt(var+eps)*w)[lbl] + (b - mean*rsqrt(var+eps)*w)[lbl]

Sharding: data-parallel over batch B across the 8 cores (64 batches each).
Each core streams its x shard once to accumulate per-(class, feature)
sum / sum-of-squares, the tiny [16, 128] partials are AllReduced on-device,
scale/shift are computed and gathered per batch with small matmuls against
the one-hot label mask, and a second streaming pass applies the affine.
"""

import sys

if "/opt/trn_rl_repo" not in sys.path:
    sys.path.insert(0, "/opt/trn_rl_repo")

import numpy as np

import concourse.bacc as bacc
import concourse.tile as tile
from concourse import mybir
from concourse import bass_utils

B, F, L = 512, 128, 1024
K = 8
N_CORES = 8
B_LOC = B // N_CORES  # 64
EPS = 1e-5

F32 = mybir.dt.float32
AFT = mybir.ActivationFunctionType

_built = None


def _build():
    nc = bacc.Bacc("TRN2", target_bir_lowering=False, debug=False,
                   num_devices=N_CORES)

    x = nc.dram_tensor("x", [B_LOC, F, L], F32, kind="ExternalInput")
    # One-hot label mask, transposed: maskT[k, j] = 1 iff labels[shard j] == k
    maskT = nc.dram_tensor("maskT", [K, B_LOC], F32, kind="ExternalInput")
    # Block-diagonal mask for the stats matmul: mask2[j, k] = maskT[k, j] and
    # mask2[64+j, 8+k] = maskT[k, j] (sum half / sum-of-squares half).
    mask2 = nc.dram_tensor("mask2", [2 * B_LOC, 2 * K], F32,
                           kind="ExternalInput")
    ident = nc.dram_tensor("ident", [128, 128], F32, kind="ExternalInput")
    rcp_cnt = nc.dram_tensor("rcp_cnt", [K, 1], F32, kind="ExternalInput")
    epsv = nc.dram_tensor("epsv", [K, 1], F32, kind="ExternalInput")
    weight = nc.dram_tensor("weight", [K, F], F32, kind="ExternalInput")
    bias = nc.dram_tensor("bias", [K, F], F32, kind="ExternalInput")
    y = nc.dram_tensor("y", [B_LOC, F, L], F32, kind="ExternalOutput")

    # Last RES batches of the shard stay resident in SBUF after pass 1 so
    # pass 2 needn't re-read them from HBM (saves RES*0.5 MiB of traffic).
    RES = 20

    with tile.TileContext(nc) as tc:
        with (
            tc.tile_pool(name="const", bufs=1) as constp,
            tc.tile_pool(name="xin", bufs=8) as xin,
            tc.tile_pool(name="xres", bufs=RES) as xres,
            tc.tile_pool(name="stats", bufs=1) as statsp,
            tc.tile_pool(name="psum", bufs=1, space="PSUM") as psum,
            tc.tile_pool(name="dram", bufs=1, space="DRAM") as dram,
            tc.tile_pool(name="xin2", bufs=13) as xin2,
            tc.tile_pool(name="yout", bufs=3) as yout,
        ):
            identt = constp.tile([128, 128], F32)
            nc.sync.dma_start(identt[:], ident[:])
            mask2t = constp.tile([2 * B_LOC, 2 * K], F32)
            nc.sync.dma_start(mask2t[:], mask2[:])
            maskTt = constp.tile([K, B_LOC], F32)
            nc.sync.dma_start(maskTt[:], maskT[:])
            rcpt = constp.tile([K, 1], F32)
            nc.sync.dma_start(rcpt[:], rcp_cnt[:])
            epst = constp.tile([K, 1], F32)
            nc.sync.dma_start(epst[:], epsv[:])
            wt = constp.tile([K, F], F32)
            nc.sync.dma_start(wt[:], weight[:])
            bt = constp.tile([K, F], F32)
            nc.sync.dma_start(bt[:], bias[:])

            # ---- pass 1: per-batch row sums / sums of squares ----
            # S[:, b] = sum_l x[b, :, l] (DVE); Q[:, b] = sum_l x[b,:,l]^2
            # (ACT).  Separate S/Q tiles: a shared tile would make Tile
            # serialize the two engines on false WAW sharing.
            S = statsp.tile([128, B_LOC], F32)
            Q = statsp.tile([128, B_LOC], F32)
            scratch_a = statsp.tile([128, L], F32)
            res_tiles = {}
            for b in range(B_LOC):
                if b >= B_LOC - RES:
                    xt = xres.tile([F, L], F32)
                    res_tiles[b] = xt
                else:
                    xt = xin.tile([F, L], F32)
                nc.sync.dma_start(xt[:], x[b])
                nc.scalar.activation(scratch_a[:], xt[:], AFT.Square,
                                     accum_out=Q[:, b:b + 1])
                nc.vector.reduce_sum(S[:, b:b + 1], xt[:],
                                     axis=mybir.AxisListType.X)

            # ---- per-class reduction: transpose + masked matmul ----
            # sqt partitions 0..63 = S^T (batch-major), 64..127 = Q^T.
            st_ps = psum.tile([B_LOC, 128], F32)
            nc.tensor.transpose(st_ps[:], S[:], identt[:])
            qt_ps = psum.tile([B_LOC, 128], F32)
            nc.tensor.transpose(qt_ps[:], Q[:], identt[:])
            sqt = statsp.tile([128, 128], F32)
            nc.vector.tensor_copy(sqt[0:B_LOC, :], st_ps[:])
            nc.vector.tensor_copy(sqt[B_LOC:128, :], qt_ps[:])

            part_ps = psum.tile([2 * K, 128], F32)
            nc.tensor.matmul(part_ps[:], mask2t[:], sqt[:], start=True,
                             stop=True)
            part = statsp.tile([2 * K, 128], F32)
            nc.vector.tensor_copy(part[:], part_ps[:])

            # ---- all-reduce the [16, 128] partials across the 8 cores ----
            cc_in = dram.tile([2 * K, 128], F32)
            cc_out = dram.tile([2 * K, 128], F32)
            nc.sync.dma_start(cc_in[:], part[:])
            nc.gpsimd.collective_compute(
                "AllReduce",
                mybir.AluOpType.add,
                replica_groups=[list(range(N_CORES))],
                ins=[cc_in.opt()],
                outs=[cc_out.opt()],
            )
            # G loads issue from the ACT sequencer: they must wait for the
            # AllReduce, and a wait on the in-order Sync stream would block
            # the pass-2 prefetch issues queued behind it.
            Gs = statsp.tile([K, 128], F32)
            nc.scalar.dma_start(Gs[:], cc_out[0:K])
            Gq = statsp.tile([K, 128], F32)
            nc.scalar.dma_start(Gq[:], cc_out[K:2 * K])

            # ---- scale/shift per (class, feature) ----
            mean = statsp.tile([K, F], F32)
            nc.vector.tensor_scalar_mul(mean[:], Gs[:], rcpt[:])
            msq = statsp.tile([K, F], F32)
            nc.vector.tensor_scalar_mul(msq[:], Gq[:], rcpt[:])
            var = statsp.tile([K, F], F32)
            nc.vector.tensor_mul(var[:], mean[:], mean[:])
            nc.vector.tensor_sub(var[:], msq[:], var[:])
            std = statsp.tile([K, F], F32)
            nc.scalar.activation(std[:], var[:], AFT.Sqrt, bias=epst[:])
            inv = statsp.tile([K, F], F32)
            nc.vector.reciprocal(inv[:], std[:])
            scal = statsp.tile([K, F], F32)
            nc.vector.tensor_mul(scal[:], inv[:], wt[:])
            shft = statsp.tile([K, F], F32)
            nc.vector.tensor_mul(shft[:], mean[:], scal[:])
            nc.vector.tensor_sub(shft[:], bt[:], shft[:])

            # ---- select per-batch scale/shift columns: [F, B_LOC] ----
            ssel_ps = psum.tile([F, B_LOC], F32)
            nc.tensor.matmul(ssel_ps[:], scal[:], maskTt[:], start=True,
                             stop=True)
            ssel = statsp.tile([F, B_LOC], F32)
            nc.vector.tensor_copy(ssel[:], ssel_ps[:])
            tsel_ps = psum.tile([F, B_LOC], F32)
            nc.tensor.matmul(tsel_ps[:], shft[:], maskTt[:], start=True,
                             stop=True)
            tsel = statsp.tile([F, B_LOC], F32)
            nc.vector.tensor_copy(tsel[:], tsel_ps[:])

            # ---- pass 2: y[b] = x[b] * ssel[:, b] + tsel[:, b] ----
            # Resident batches first: their applies are ready the moment
            # ssel/tsel land, keeping the store stream busy while the
            # remaining batches re-load. Applies alternate ACT/DVE.
            def apply(b, xt_ap):
                yt = yout.tile([F, L], F32)
                if b % 2 == 0:
                    nc.scalar.activation(yt[:], xt_ap, AFT.Identity,
                                         bias=tsel[:, b:b + 1],
                                         scale=ssel[:, b:b + 1])
                else:
                    nc.vector.tensor_scalar(yt[:], xt_ap,
                                            ssel[:, b:b + 1],
                                            tsel[:, b:b + 1],
                                            mybir.AluOpType.mult,
                                            mybir.AluOpType.add)
                nc.gpsimd.dma_start(y[b], yt[:])

            for b in range(B_LOC - RES, B_LOC):
                apply(b, res_tiles[b][:])
            for b in range(B_LOC - RES):
                xt2 = xin2.tile([F, L], F32)
                nc.sync.dma_start(xt2[:], x[b])
                apply(b, xt2[:])

    nc.finalize()
    return nc


def _get_nc():
    global _built
    if _built is None:
        _built = _build()
    return _built


def _host_inputs(x, labels, weight, bias):
    labels = np.asarray(labels).astype(np.int64)
    counts = np.bincount(labels, minlength=K).astype(np.float64) * L
    rcp = (1.0 / np.maximum(counts, 1.0)).astype(np.float32).reshape(K, 1)
    ident = np.eye(128, dtype=np.float32)

    in_maps = []
    for c in range(N_CORES):
        lab = labels[c * B_LOC:(c + 1) * B_LOC]
        maskT = np.zeros((K, B_LOC), dtype=np.float32)
        maskT[lab, np.arange(B_LOC)] = 1.0
        mask2 = np.zeros((2 * B_LOC, 2 * K), dtype=np.float32)
        mask2[:B_LOC, :K] = maskT.T
        mask2[B_LOC:, K:] = maskT.T
        in_maps.append({
            "x": np.ascontiguousarray(x[c * B_LOC:(c + 1) * B_LOC]),
            "maskT": maskT,
            "mask2": mask2,
            "ident": ident,
            "rcp_cnt": rcp,
            "epsv": np.full((K, 1), EPS, dtype=np.float32),
            "weight": np.ascontiguousarray(weight.astype(np.float32)),
            "bias": np.ascontiguousarray(bias.astype(np.float32)),
        })
    return in_maps


def run(x, labels, weight, bias, trace=False):
    nc = _get_nc()
    in_maps = _host_inputs(x, labels, weight, bias)
    res = bass_utils.run_bass_kernel_spmd(nc, in_maps, list(range(N_CORES)),
                                          trace=trace)
    out = np.concatenate([res.results[c]["y"] for c in range(N_CORES)],
                         axis=0)
    return out, res


def kernel(x, labels, weight, bias):
    out, _ = run(np.asarray(x, dtype=np.float32), labels,
                 np.asarray(weight, dtype=np.float32),
                 np.asarray(bias, dtype=np.float32))
    return out


# revision 21
# speedup vs baseline: 1.0364x; 1.0364x over previous
"""Conditional BatchNorm1d (training-mode, per-class stats) on 8 Trainium2
NeuronCores.

Problem: x [512, 128, 1024] f32, labels [512] i32 in [0,8), weight/bias
[8, 128] f32.  Per-class biased mean/var over the class's (batch, length)
elements per feature, then per-class affine:
    y = x * (rsqrt(var+eps)*w)[lbl] + (b - mean*rsqrt(var+eps)*w)[lbl]

Sharding: data-parallel over batch B across the 8 cores (64 batches each).
Each core streams its x shard once to accumulate per-(class, feature)
sum / sum-of-squares, the tiny [16, 128] partials are AllReduced on-device,
scale/shift are computed and gathered per batch with small matmuls against
the one-hot label mask, and a second streaming pass applies the affine.
"""

import sys

if "/opt/trn_rl_repo" not in sys.path:
    sys.path.insert(0, "/opt/trn_rl_repo")

import numpy as np

import concourse.bacc as bacc
import concourse.tile as tile
from concourse import mybir
from concourse import bass_utils

B, F, L = 512, 128, 1024
K = 8
N_CORES = 8
B_LOC = B // N_CORES  # 64
EPS = 1e-5

F32 = mybir.dt.float32
AFT = mybir.ActivationFunctionType

_built = None


def _build():
    nc = bacc.Bacc("TRN2", target_bir_lowering=False, debug=False,
                   num_devices=N_CORES)

    x = nc.dram_tensor("x", [B_LOC, F, L], F32, kind="ExternalInput")
    # One-hot label mask, transposed: maskT[k, j] = 1 iff labels[shard j] == k
    maskT = nc.dram_tensor("maskT", [K, B_LOC], F32, kind="ExternalInput")
    # Block-diagonal mask for the stats matmul: mask2[j, k] = maskT[k, j] and
    # mask2[64+j, 8+k] = maskT[k, j] (sum half / sum-of-squares half).
    mask2 = nc.dram_tensor("mask2", [2 * B_LOC, 2 * K], F32,
                           kind="ExternalInput")
    ident = nc.dram_tensor("ident", [128, 128], F32, kind="ExternalInput")
    rcp_cnt = nc.dram_tensor("rcp_cnt", [K, 1], F32, kind="ExternalInput")
    epsv = nc.dram_tensor("epsv", [K, 1], F32, kind="ExternalInput")
    weight = nc.dram_tensor("weight", [K, F], F32, kind="ExternalInput")
    bias = nc.dram_tensor("bias", [K, F], F32, kind="ExternalInput")
    y = nc.dram_tensor("y", [B_LOC, F, L], F32, kind="ExternalOutput")

    # Last RES batches of the shard stay resident in SBUF after pass 1 so
    # pass 2 needn't re-read them from HBM (saves RES*0.5 MiB of traffic).
    RES = 18

    with tile.TileContext(nc) as tc:
        with (
            tc.tile_pool(name="const", bufs=1) as constp,
            tc.tile_pool(name="xin", bufs=8) as xin,
            tc.tile_pool(name="xres", bufs=RES) as xres,
            tc.tile_pool(name="stats", bufs=1) as statsp,
            tc.tile_pool(name="psum", bufs=1, space="PSUM") as psum,
            tc.tile_pool(name="dram", bufs=1, space="DRAM") as dram,
            tc.tile_pool(name="xin2", bufs=12) as xin2,
            tc.tile_pool(name="yout", bufs=3) as yout,
        ):
            identt = constp.tile([128, 128], F32)
            nc.scalar.dma_start(identt[:], ident[:])
            mask2t = constp.tile([2 * B_LOC, 2 * K], F32)
            nc.scalar.dma_start(mask2t[:], mask2[:])
            maskTt = constp.tile([K, B_LOC], F32)
            nc.scalar.dma_start(maskTt[:], maskT[:])
            rcpt = constp.tile([K, 1], F32)
            nc.scalar.dma_start(rcpt[:], rcp_cnt[:])
            epst = constp.tile([K, 1], F32)
            nc.scalar.dma_start(epst[:], epsv[:])
            wt = constp.tile([K, F], F32)
            nc.scalar.dma_start(wt[:], weight[:])
            bt = constp.tile([K, F], F32)
            nc.scalar.dma_start(bt[:], bias[:])

            # ---- pass 1: per-batch row sums / sums of squares ----
            # S[:, b] = sum_l x[b, :, l] (DVE); Q[:, b] = sum_l x[b,:,l]^2
            # (ACT).  Separate S/Q tiles: a shared tile would make Tile
            # serialize the two engines on false WAW sharing.
            S = statsp.tile([128, B_LOC], F32)
            Q = statsp.tile([128, B_LOC], F32)
            # ACT square scratch lives in PSUM (2 banks) to save an SBUF slot
            scratch_a = psum.tile([128, L], F32)
            res_tiles = {}
            for b in range(B_LOC):
                if b >= B_LOC - RES:
                    xt = xres.tile([F, L], F32)
                    res_tiles[b] = xt
                else:
                    xt = xin.tile([F, L], F32)
                nc.sync.dma_start(xt[:], x[b])
                nc.scalar.activation(scratch_a[:], xt[:], AFT.Square,
                                     accum_out=Q[:, b:b + 1])
                nc.vector.reduce_sum(S[:, b:b + 1], xt[:],
                                     axis=mybir.AxisListType.X)

            # ---- per-class reduction: transpose + masked matmul ----
            # sqt partitions 0..63 = S^T (batch-major), 64..127 = Q^T.
            st_ps = psum.tile([B_LOC, 128], F32)
            nc.tensor.transpose(st_ps[:], S[:], identt[:])
            qt_ps = psum.tile([B_LOC, 128], F32)
            nc.tensor.transpose(qt_ps[:], Q[:], identt[:])
            sqt = statsp.tile([128, 128], F32)
            nc.vector.tensor_copy(sqt[0:B_LOC, :], st_ps[:])
            nc.vector.tensor_copy(sqt[B_LOC:128, :], qt_ps[:])

            part_ps = psum.tile([2 * K, 128], F32)
            nc.tensor.matmul(part_ps[:], mask2t[:], sqt[:], start=True,
                             stop=True)
            part = statsp.tile([2 * K, 128], F32)
            nc.vector.tensor_copy(part[:], part_ps[:])

            # ---- all-reduce the [16, 128] partials across the 8 cores ----
            cc_in = dram.tile([2 * K, 128], F32)
            cc_out = dram.tile([2 * K, 128], F32)
            nc.sync.dma_start(cc_in[:], part[:])
            nc.gpsimd.collective_compute(
                "AllReduce",
                mybir.AluOpType.add,
                replica_groups=[list(range(N_CORES))],
                ins=[cc_in.opt()],
                outs=[cc_out.opt()],
            )
            # G loads issue from the ACT sequencer: they must wait for the
            # AllReduce, and a wait on the in-order Sync stream would block
            # the pass-2 prefetch issues queued behind it.
            Gs = statsp.tile([K, 128], F32)
            nc.scalar.dma_start(Gs[:], cc_out[0:K])
            Gq = statsp.tile([K, 128], F32)
            nc.scalar.dma_start(Gq[:], cc_out[K:2 * K])

            # ---- scale/shift per (class, feature) ----
            mean = statsp.tile([K, F], F32)
            nc.vector.tensor_scalar_mul(mean[:], Gs[:], rcpt[:])
            msq = statsp.tile([K, F], F32)
            nc.vector.tensor_scalar_mul(msq[:], Gq[:], rcpt[:])
            var = statsp.tile([K, F], F32)
            nc.vector.tensor_mul(var[:], mean[:], mean[:])
            nc.vector.tensor_sub(var[:], msq[:], var[:])
            std = statsp.tile([K, F], F32)
            nc.scalar.activation(std[:], var[:], AFT.Sqrt, bias=epst[:])
            inv = statsp.tile([K, F], F32)
            nc.vector.reciprocal(inv[:], std[:])
            scal = statsp.tile([K, F], F32)
            nc.vector.tensor_mul(scal[:], inv[:], wt[:])
            shft = statsp.tile([K, F], F32)
            nc.vector.tensor_mul(shft[:], mean[:], scal[:])
            nc.vector.tensor_sub(shft[:], bt[:], shft[:])

            # ---- select per-batch scale/shift columns: [F, B_LOC] ----
            ssel_ps = psum.tile([F, B_LOC], F32)
            nc.tensor.matmul(ssel_ps[:], scal[:], maskTt[:], start=True,
                             stop=True)
            ssel = statsp.tile([F, B_LOC], F32)
            nc.vector.tensor_copy(ssel[:], ssel_ps[:])
            tsel_ps = psum.tile([F, B_LOC], F32)
            nc.tensor.matmul(tsel_ps[:], shft[:], maskTt[:], start=True,
                             stop=True)
            tsel = statsp.tile([F, B_LOC], F32)
            nc.vector.tensor_copy(tsel[:], tsel_ps[:])

            # ---- pass 2: y[b] = x[b] * ssel[:, b] + tsel[:, b] ----
            # Resident batches first: their applies are ready the moment
            # ssel/tsel land, keeping the store stream busy while the
            # remaining batches re-load. Applies alternate ACT/DVE.
            def apply(b, xt_ap):
                yt = yout.tile([F, L], F32)
                if b % 2 == 0:
                    nc.scalar.activation(yt[:], xt_ap, AFT.Identity,
                                         bias=tsel[:, b:b + 1],
                                         scale=ssel[:, b:b + 1])
                else:
                    nc.vector.tensor_scalar(yt[:], xt_ap,
                                            ssel[:, b:b + 1],
                                            tsel[:, b:b + 1],
                                            mybir.AluOpType.mult,
                                            mybir.AluOpType.add)
                nc.gpsimd.dma_start(y[b], yt[:])

            for b in range(B_LOC - RES, B_LOC):
                apply(b, res_tiles[b][:])
            for b in range(B_LOC - RES):
                # Reuse the (now idle) pass-1 xin slots for the first few
                # reloads: extends prefetch depth over the AllReduce window
                # at zero extra SBUF cost.
                pool = xin if b < 8 else xin2
                xt2 = pool.tile([F, L], F32)
                nc.sync.dma_start(xt2[:], x[b])
                apply(b, xt2[:])

    nc.finalize()
    return nc


def _get_nc():
    global _built
    if _built is None:
        _built = _build()
    return _built


def _host_inputs(x, labels, weight, bias):
    labels = np.asarray(labels).astype(np.int64)
    counts = np.bincount(labels, minlength=K).astype(np.float64) * L
    rcp = (1.0 / np.maximum(counts, 1.0)).astype(np.float32).reshape(K, 1)
    ident = np.eye(128, dtype=np.float32)

    in_maps = []
    for c in range(N_CORES):
        lab = labels[c * B_LOC:(c + 1) * B_LOC]
        maskT = np.zeros((K, B_LOC), dtype=np.float32)
        maskT[lab, np.arange(B_LOC)] = 1.0
        mask2 = np.zeros((2 * B_LOC, 2 * K), dtype=np.float32)
        mask2[:B_LOC, :K] = maskT.T
        mask2[B_LOC:, K:] = maskT.T
        in_maps.append({
            "x": np.ascontiguousarray(x[c * B_LOC:(c + 1) * B_LOC]),
            "maskT": maskT,
            "mask2": mask2,
            "ident": ident,
            "rcp_cnt": rcp,
            "epsv": np.full((K, 1), EPS, dtype=np.float32),
            "weight": np.ascontiguousarray(weight.astype(np.float32)),
            "bias": np.ascontiguousarray(bias.astype(np.float32)),
        })
    return in_maps


def run(x, labels, weight, bias, trace=False):
    nc = _get_nc()
    in_maps = _host_inputs(x, labels, weight, bias)
    res = bass_utils.run_bass_kernel_spmd(nc, in_maps, list(range(N_CORES)),
                                          trace=trace)
    out = np.concatenate([res.results[c]["y"] for c in range(N_CORES)],
                         axis=0)
    return out, res


def kernel(x, labels, weight, bias):
    out, _ = run(np.asarray(x, dtype=np.float32), labels,
                 np.asarray(weight, dtype=np.float32),
                 np.asarray(bias, dtype=np.float32))
    return out
